# revision 1
# baseline (speedup 1.0000x reference)
"""MoE experts kernel for Trainium2 (8 NeuronCores, expert-parallel).

Reference computation (per token t, top-k expert e with gate p):
    y[t] = sum_k p[t,k] * down_e @ (silu(x[t] @ gate_e) * (x[t] @ up_e))
with per-expert capacity CAP=1024 (tokens beyond capacity dropped).

Strategy:
  - Host: sort token assignments by expert (stable, matching jnp.argsort),
    build per-expert dense token buffers transposed to [128, NH, w] so every
    device DMA is contiguous.
  - Experts ranked by load; expert ranked r -> core r%8, slot r//8, so each
    slot's compile-time width (cross-core max, rounded to 8) hugs the actual
    loads: sum(slotw) ~ 4176 vs 4096 ideal (~2% padding).
  - Device (SPMD over 8 cores, 8 experts/core): grouped GEMMs in fp16
    (full-rate PE, fp32 PSUM accumulation):
       G^T = gate^T-slices @ X^T   (accumulate over H chunks)
       U^T = up^T-slices   @ X^T
       Hm^T = silu(G^T) * U^T
       O^T  = down^T-slices @ Hm^T (accumulate over I chunks)
    plus a PE warmup block (dummy matmuls) overlapping the initial DMA so the
    HAM clock gate is released before real matmuls start; output in fp16.
  - Host: gather rows back, apply routing weights, sum over top-k in fp32.
"""

import os
import sys

sys.path.insert(0, "/opt/trn_rl_repo")

import numpy as np

E, H, I, T, K = 64, 2048, 768, 4096, 8
CAP = 1024
NCORES = 8
EPC = E // NCORES  # experts per core
NH = H // 128  # 16 contraction chunks for gate/up
NI = I // 128  # 6 contraction chunks for down
WARMUP_MM = 56  # dummy matmuls to release the HAM clock gate during DMA lead-in

_prog_cache = {}
LAST_EXEC_NS = None
LAST_RESULTS = None


def _groups(npad):
    ng = -(-npad // 512)
    w = -(-npad // ng)
    out = []
    s = 0
    while s < npad:
        e = min(s + w, npad)
        out.append((s, e))
        s = e
    return out


def _build_program(slotw):
    import concourse.bacc as bacc
    import concourse.mybir as mybir
    from concourse.tile import TileContext

    f32 = mybir.dt.float32
    f16 = mybir.dt.float16
    SILU = mybir.ActivationFunctionType.Silu

    nc = bacc.Bacc(None, target_bir_lowering=False)
    wz = nc.declare_dram_parameter("wz", [128, 64], f16, isOutput=False)
    xTs = [
        nc.declare_dram_parameter(f"xT{j}", [128, NH, w], f16, isOutput=False)
        for j, w in enumerate(slotw)
    ]
    gw = nc.declare_dram_parameter("gw", [EPC, NI, 128, NH, 128], f16, isOutput=False)
    uw = nc.declare_dram_parameter("uw", [EPC, NI, 128, NH, 128], f16, isOutput=False)
    dw = nc.declare_dram_parameter("dw", [EPC, 128, NH, NI, 128], f16, isOutput=False)
    yTs = [
        nc.declare_dram_parameter(f"yT{j}", [128, NH, w], f16, isOutput=True)
        for j, w in enumerate(slotw)
    ]

    with TileContext(nc) as tc:
        with (
            tc.sbuf_pool(name="xp", bufs=2) as xp,
            tc.sbuf_pool(name="wp", bufs=3) as wp,
            tc.sbuf_pool(name="hp", bufs=1) as hp,
            tc.sbuf_pool(name="op", bufs=2) as op,
            tc.sbuf_pool(name="tp", bufs=3) as tp,
            tc.psum_pool(name="pp", bufs=2) as pp,
        ):
            # --- PE warmup: keep the tensor engine busy while the first real
            # DMAs land so the HAM clock gate opens to 8/8 before real MMs.
            wzt = wp.tile([128, 64], f16, name="wz", tag="wz", bufs=1)
            nc.vector.memset(wzt, 0)
            pw = pp.tile([64, 64], f32, name="pw", tag="pw", bufs=1)
            for _ in range(WARMUP_MM):
                nc.tensor.matmul(pw, wzt[:, :64], wzt, start=True, stop=True)

            for e in range(EPC):
                npad = slotw[e]
                groups = _groups(npad)
                if e == 0:
                    # Fine-grained first loads spread over parallel DMA queues
                    # so the first real matmul can start as soon as possible
                    # (cold DMA queues run at ~1/4 rate).
                    ga = wp.tile([128, 8, 128], f16, name="g_w", tag="g_w")
                    gb = wp.tile([128, 8, 128], f16, name="g_w", tag="g_w")
                    ua = wp.tile([128, 8, 128], f16, name="u_w", tag="u_w")
                    ub = wp.tile([128, 8, 128], f16, name="u_w", tag="u_w")
                    xt_tiles = [
                        xp.tile([128, 2, npad], f16, name=f"xt{j}", tag=f"xt{j % 4}")
                        for j in range(8)
                    ]
                    nc.sync.dma_start(out=ga, in_=gw[e, 0, :, 0:8, :])
                    nc.sync.dma_start(out=xt_tiles[0], in_=xTs[e][:, 0:2, :])
                    nc.sync.dma_start(out=gb, in_=gw[e, 0, :, 8:16, :])
                    nc.sync.dma_start(out=xt_tiles[1], in_=xTs[e][:, 2:4, :])
                    nc.sync.dma_start(out=xt_tiles[2], in_=xTs[e][:, 4:6, :])
                    nc.sync.dma_start(out=xt_tiles[3], in_=xTs[e][:, 6:8, :])
                    nc.sync.dma_start(out=ua, in_=uw[e, 0, :, 0:8, :])
                    nc.sync.dma_start(out=xt_tiles[4], in_=xTs[e][:, 8:10, :])
                    nc.sync.dma_start(out=xt_tiles[5], in_=xTs[e][:, 10:12, :])
                    nc.sync.dma_start(out=ub, in_=uw[e, 0, :, 8:16, :])
                    nc.sync.dma_start(out=xt_tiles[6], in_=xTs[e][:, 12:14, :])
                    nc.sync.dma_start(out=xt_tiles[7], in_=xTs[e][:, 14:16, :])
                    xts = [xt_tiles[h // 2][:, h % 2, :] for h in range(NH)]
                    g0_aps = [(ga if h < 8 else gb)[:, h % 8, :] for h in range(NH)]
                    u0_aps = [(ua if h < 8 else ub)[:, h % 8, :] for h in range(NH)]
                else:
                    g_w0 = wp.tile([128, NH, 128], f16, name="g_w", tag="g_w")
                    u_w0 = wp.tile([128, NH, 128], f16, name="u_w", tag="u_w")
                    xts = []
                    nc.sync.dma_start(out=g_w0, in_=gw[e, 0, :, :, :])
                    for j in range(4):
                        xt_t = xp.tile([128, 4, npad], f16, name=f"xt{j}", tag=f"xt{j}")
                        nc.sync.dma_start(
                            out=xt_t, in_=xTs[e][:, 4 * j : 4 * (j + 1), :]
                        )
                        xts.extend(xt_t[:, jj, :] for jj in range(4))
                        if j == 0:
                            nc.sync.dma_start(out=u_w0, in_=uw[e, 0, :, :, :])
                    g0_aps = [g_w0[:, h, :] for h in range(NH)]
                    u0_aps = [u_w0[:, h, :] for h in range(NH)]
                hms = [
                    hp.tile([128, npad], f16, name=f"hm{i}", tag=f"hm{i}")
                    for i in range(NI)
                ]
                for i in range(NI):
                    if i == 0:
                        g_aps, u_aps = g0_aps, u0_aps
                    else:
                        g_w = wp.tile([128, NH, 128], f16, name="g_w", tag="g_w")
                        u_w = wp.tile([128, NH, 128], f16, name="u_w", tag="u_w")
                        nc.sync.dma_start(out=g_w, in_=gw[e, i, :, :, :])
                        nc.sync.dma_start(out=u_w, in_=uw[e, i, :, :, :])
                        g_aps = [g_w[:, h, :] for h in range(NH)]
                        u_aps = [u_w[:, h, :] for h in range(NH)]
                    if e == 0 and i == 0 and len(groups) == 2:
                        # Interleave the two groups' accumulation sweeps so the
                        # first expert consumes X chunks at half rate, matching
                        # the cold DMA queues' delivery pace.
                        psgs = [
                            pp.tile([128, g1 - g0, ], f32, name="psg", tag="psg", bufs=3)
                            for g0, g1 in groups
                        ]
                        psus = [
                            pp.tile([128, g1 - g0], f32, name="psu", tag="psu", bufs=2)
                            for g0, g1 in groups
                        ]
                        for h in range(NH):
                            for gi, (g0, g1) in enumerate(groups):
                                nc.tensor.matmul(
                                    psgs[gi],
                                    g_aps[h],
                                    xts[h][:, g0:g1],
                                    start=(h == 0),
                                    stop=(h == NH - 1),
                                )
                        for h in range(NH):
                            for gi, (g0, g1) in enumerate(groups):
                                nc.tensor.matmul(
                                    psus[gi],
                                    u_aps[h],
                                    xts[h][:, g0:g1],
                                    start=(h == 0),
                                    stop=(h == NH - 1),
                                )
                        for gi, (g0, g1) in enumerate(groups):
                            sil = tp.tile([128, g1 - g0], f32, name="sil", tag="sil")
                            nc.scalar.activation(sil, psgs[gi], SILU)
                            nc.vector.tensor_mul(hms[i][:, g0:g1], sil, psus[gi])
                        continue_groups = False
                    else:
                        continue_groups = True
                    if continue_groups:
                        for g0, g1 in groups:
                            wdt = g1 - g0
                            psg = pp.tile([128, wdt], f32, name="psg", tag="psg", bufs=3)
                            psu = pp.tile([128, wdt], f32, name="psu", tag="psu", bufs=2)
                            for h in range(NH):
                                nc.tensor.matmul(
                                    psg,
                                    g_aps[h],
                                    xts[h][:, g0:g1],
                                    start=(h == 0),
                                    stop=(h == NH - 1),
                                )
                            for h in range(NH):
                                nc.tensor.matmul(
                                    psu,
                                    u_aps[h],
                                    xts[h][:, g0:g1],
                                    start=(h == 0),
                                    stop=(h == NH - 1),
                                )
                            sil = tp.tile([128, wdt], f32, name="sil", tag="sil")
                            nc.scalar.activation(sil, psg, SILU)
                            nc.vector.tensor_mul(hms[i][:, g0:g1], sil, psu)
                d_w = wp.tile([128, NH, NI, 128], f16, name="d_w", tag="d_w", bufs=2)
                nc.sync.dma_start(out=d_w, in_=dw[e])
                for q in range(4):
                    yt = op.tile([128, 4, npad], f16, name=f"yt{q}", tag=f"yt{q}")
                    for hh in range(4):
                        h = 4 * q + hh
                        for gi, (g0, g1) in enumerate(groups):
                            wdt = g1 - g0
                            pso = pp.tile([128, wdt], f32, name="pso", tag="pso")
                            for i in range(NI):
                                nc.tensor.matmul(
                                    pso,
                                    d_w[:, h, i, :],
                                    hms[i][:, g0:g1],
                                    start=(i == 0),
                                    stop=(i == NI - 1),
                                )
                            nc.vector.tensor_copy(yt[:, hh, g0:g1], pso)
                    nc.sync.dma_start(out=yTs[e][:, 4 * q : 4 * (q + 1), :], in_=yt)
    nc.compile()
    return nc


def _install_neff_cache():
    """Cache walrus NEFF compiles on disk keyed by BIR hash (compile of the
    ~11k-instruction program takes minutes; the BIR is deterministic)."""
    import hashlib
    import shutil

    import concourse.bass2jax as bass2jax
    from concourse.bass_utils import compile_bir_kernel as _orig

    if getattr(bass2jax.compile_bir_kernel, "_moe_cached", False):
        return
    cache_dir = os.environ.get("BASS_NEFF_CACHE", "/tmp/bass_neff_cache")
    try:
        os.makedirs(cache_dir, exist_ok=True)
    except OSError:
        return

    def cached(bir_json, tmpdir, neff_name="file.neff"):
        key = hashlib.sha256(bir_json).hexdigest()[:24]
        cpath = os.path.join(cache_dir, key + ".neff")
        dst = os.path.join(tmpdir, neff_name)
        if os.path.exists(cpath):
            shutil.copy(cpath, dst)
            return dst
        out = _orig(bir_json, tmpdir, neff_name)
        try:
            shutil.copy(out, cpath)
        except OSError:
            pass
        return out

    cached._moe_cached = True
    bass2jax.compile_bir_kernel = cached


def _install_ntff_hook_shim():
    """Provide antenv.axon_hooks (absent in this container) so that
    run_bass_kernel_spmd(trace=True) can capture NTFF profiles via the
    axon .so — mirrors trn_agent_boot.trn_boot's ctypes hook."""
    import types
    import ctypes
    import contextlib

    if "antenv.axon_hooks" in sys.modules:
        return
    so_path = "/opt/axon/libaxon_pjrt.so"
    lib = ctypes.CDLL(so_path)
    if not hasattr(lib, "axon_start_nrt_profile"):
        return
    lib.axon_start_nrt_profile.argtypes = [
        ctypes.POINTER(ctypes.c_int64),
        ctypes.c_size_t,
    ]
    lib.axon_start_nrt_profile.restype = ctypes.c_int64
    lib.axon_stop_nrt_profile.argtypes = [ctypes.c_char_p]
    lib.axon_stop_nrt_profile.restype = ctypes.c_int64

    @contextlib.contextmanager
    def _hook(output_dir, device_ids):
        import jax

        jax.devices()
        if device_ids:
            ids = (ctypes.c_int64 * len(device_ids))(*device_ids)
            rc = lib.axon_start_nrt_profile(ids, len(device_ids))
        else:
            rc = lib.axon_start_nrt_profile(None, 0)
        if rc != 0:
            raise RuntimeError(f"axon_start_nrt_profile rc={rc}")
        try:
            yield
        finally:
            n = lib.axon_stop_nrt_profile(str(output_dir).encode())
            print(f"profile: {n} file(s) written to {output_dir}", file=sys.stderr)

    mod = types.ModuleType("antenv.axon_hooks")
    mod.get_axon_ntff_profile_hook = lambda: _hook
    mod.set_axon_ntff_profile_hook = lambda h: None
    sys.modules["antenv.axon_hooks"] = mod


def kernel(
    hidden_states,
    routing_weights,
    selected_experts,
    gate_proj,
    up_proj,
    down_proj,
):
    global LAST_EXEC_NS, LAST_RESULTS
    from concourse.bass_utils import run_bass_kernel_spmd

    _install_neff_cache()

    x = np.ascontiguousarray(np.asarray(hidden_states, dtype=np.float32))
    rw = np.asarray(routing_weights, dtype=np.float32)
    sel = np.asarray(selected_experts).astype(np.int64)
    gate = np.asarray(gate_proj, dtype=np.float32)
    up = np.asarray(up_proj, dtype=np.float32)
    down = np.asarray(down_proj, dtype=np.float32)

    # ---- host dispatch (mirrors reference's stable sort-by-expert) ----
    flat_e = sel.reshape(-1)
    order = np.argsort(flat_e, kind="stable")
    sorted_e = flat_e[order]
    counts = np.bincount(flat_e, minlength=E)
    offsets = np.concatenate([[0], np.cumsum(counts)[:-1]])
    pos = np.arange(flat_e.shape[0], dtype=np.int64) - offsets[sorted_e]

    # ---- slot assignment: experts ranked by load desc; rank r -> core r%8,
    # slot r//8; slot width = rounded max count in its rank group ----
    rank = np.argsort(-counts, kind="stable")
    perm = rank.reshape(EPC, NCORES).T  # perm[c, j] = expert id
    slotw = tuple(
        int(min(CAP, max(16, -(-int(counts[perm[:, j]].max()) // 4) * 4)))
        for j in range(EPC)
    )
    w_of_expert = np.zeros(E, dtype=np.int64)
    for c in range(NCORES):
        for j in range(EPC):
            w_of_expert[perm[c, j]] = slotw[j]

    keep = pos < w_of_expert[sorted_e]  # width >= min(count, CAP); drops only > CAP

    tok = order // K
    ke = sorted_e[keep]
    kp = pos[keep]

    # Dense per-expert buffers, transposed: xbuf[e] = X_e  [w_e, H]
    maxw = max(slotw)
    xbuf = np.zeros((E, maxw, H), dtype=np.float32)
    xbuf[ke, kp] = x[tok[keep]]

    # ---- weight layouts (contiguous per-DMA blocks) ----
    # gate/up slice for (e, i): [128p, NH, 128c] where [p, h, c] = W[h*128+p, i*128+c]
    gate_r = gate.reshape(E, NH, 128, NI, 128).transpose(0, 3, 2, 1, 4)
    up_r = up.reshape(E, NH, 128, NI, 128).transpose(0, 3, 2, 1, 4)
    # down block for e: [128p, NH, NI, 128m] where [p, h, i, m] = W[i*128+p, h*128+m]
    down_r = down.reshape(E, NI, 128, NH, 128).transpose(0, 2, 3, 1, 4)

    nc = _prog_cache.get(slotw)
    if nc is None:
        nc = _build_program(slotw)
        _prog_cache[slotw] = nc

    in_maps = []
    for c in range(NCORES):
        m = {
            "wz": np.zeros((128, 64), dtype=np.float16),
            "gw": np.ascontiguousarray(gate_r[perm[c]], dtype=np.float16),
            "uw": np.ascontiguousarray(up_r[perm[c]], dtype=np.float16),
            "dw": np.ascontiguousarray(down_r[perm[c]], dtype=np.float16),
        }
        for j in range(EPC):
            e = perm[c, j]
            w = slotw[j]
            # [w, H] -> [128p, NH, w]
            m[f"xT{j}"] = np.ascontiguousarray(
                xbuf[e, :w].T.reshape(NH, 128, w).transpose(1, 0, 2),
                dtype=np.float16,
            )
        in_maps.append(m)

    trace = bool(os.environ.get("BASS_MOE_TRACE"))
    kwargs = {}
    if trace:
        _install_ntff_hook_shim()
        tcores = os.environ.get("BASS_MOE_TRACE_CORES", "0")
        kwargs = dict(trace=True, trace_cores=[int(c) for c in tcores.split(",")])
    res = run_bass_kernel_spmd(nc, in_maps, core_ids=list(range(NCORES)), **kwargs)
    LAST_EXEC_NS = res.exec_time_ns
    LAST_RESULTS = res

    # ---- host combine ----
    # per expert e at (core c, slot j): yT{j} is [128p, NH, w] = O_e^T blocks
    o_all = np.zeros((E, maxw, H), dtype=np.float32)
    for c in range(NCORES):
        for j in range(EPC):
            e = perm[c, j]
            w = slotw[j]
            o_all[e, :w] = (
                res.results[c][f"yT{j}"]
                .astype(np.float32)
                .transpose(2, 1, 0)
                .reshape(w, H)
            )

    gathered = np.zeros((flat_e.shape[0], H), dtype=np.float32)
    gathered[order[keep]] = o_all[ke, kp]
    y = (gathered.reshape(T, K, H) * rw[:, :, None]).sum(axis=1, dtype=np.float32)
    return y.astype(np.float32)



# revision 5
# speedup vs baseline: 1.0123x; 1.0123x over previous
"""MoE experts kernel for Trainium2 (8 NeuronCores, expert-parallel).

Reference computation (per token t, top-k expert e with gate p):
    y[t] = sum_k p[t,k] * down_e @ (silu(x[t] @ gate_e) * (x[t] @ up_e))
with per-expert capacity CAP=1024 (tokens beyond capacity dropped).

Strategy:
  - Host: sort token assignments by expert (stable, matching jnp.argsort),
    build per-expert dense token buffers transposed to [128, NH, w] so every
    device DMA is contiguous.
  - Experts ranked by load; expert ranked r -> core r%8, slot r//8, so each
    slot's compile-time width (cross-core max, rounded to 8) hugs the actual
    loads: sum(slotw) ~ 4176 vs 4096 ideal (~2% padding).
  - Device (SPMD over 8 cores, 8 experts/core): grouped GEMMs in fp16
    (full-rate PE, fp32 PSUM accumulation):
       G^T = gate^T-slices @ X^T   (accumulate over H chunks)
       U^T = up^T-slices   @ X^T
       Hm^T = silu(G^T) * U^T
       O^T  = down^T-slices @ Hm^T (accumulate over I chunks)
    plus a PE warmup block (dummy matmuls) overlapping the initial DMA so the
    HAM clock gate is released before real matmuls start; output in fp16.
  - Host: gather rows back, apply routing weights, sum over top-k in fp32.
"""

import os
import sys

sys.path.insert(0, "/opt/trn_rl_repo")

import numpy as np

E, H, I, T, K = 64, 2048, 768, 4096, 8
CAP = 1024
NCORES = 8
EPC = E // NCORES  # experts per core
NH = H // 128  # 16 contraction chunks for gate/up
NI = I // 128  # 6 contraction chunks for down
WARMUP_MM = 56  # dummy matmuls to release the HAM clock gate during DMA lead-in

_prog_cache = {}
LAST_EXEC_NS = None
LAST_RESULTS = None


def _groups(npad):
    ng = -(-npad // 512)
    w = -(-npad // ng)
    out = []
    s = 0
    while s < npad:
        e = min(s + w, npad)
        out.append((s, e))
        s = e
    return out


def _build_program(slotw):
    import concourse.bacc as bacc
    import concourse.mybir as mybir
    from concourse.tile import TileContext

    f32 = mybir.dt.float32
    f16 = mybir.dt.float16
    SILU = mybir.ActivationFunctionType.Silu

    nc = bacc.Bacc(None, target_bir_lowering=False)
    wz = nc.declare_dram_parameter("wz", [128, 64], f16, isOutput=False)
    xTs = [
        nc.declare_dram_parameter(f"xT{j}", [128, NH, w], f16, isOutput=False)
        for j, w in enumerate(slotw)
    ]
    gw = nc.declare_dram_parameter("gw", [EPC, NI, 128, NH, 128], f16, isOutput=False)
    uw = nc.declare_dram_parameter("uw", [EPC, NI, 128, NH, 128], f16, isOutput=False)
    dw = nc.declare_dram_parameter("dw", [EPC, 128, NH, NI, 128], f16, isOutput=False)
    yTs = [
        nc.declare_dram_parameter(f"yT{j}", [128, NH, w], f16, isOutput=True)
        for j, w in enumerate(slotw)
    ]

    with TileContext(nc) as tc:
        with (
            tc.sbuf_pool(name="xp", bufs=2) as xp,
            tc.sbuf_pool(name="wp", bufs=3) as wp,
            tc.sbuf_pool(name="hp", bufs=1) as hp,
            tc.sbuf_pool(name="op", bufs=2) as op,
            tc.sbuf_pool(name="tp", bufs=3) as tp,
            tc.psum_pool(name="pp", bufs=2) as pp,
        ):
            # --- PE warmup: keep the tensor engine busy while the first real
            # DMAs land so the HAM clock gate opens to 8/8 before real MMs.
            wzt = wp.tile([128, 64], f16, name="wz", tag="wz", bufs=1)
            nc.vector.memset(wzt, 0)
            pw = pp.tile([64, 64], f32, name="pw", tag="pw", bufs=1)
            for _ in range(WARMUP_MM):
                nc.tensor.matmul(pw, wzt[:, :64], wzt, start=True, stop=True)

            for e in range(EPC):
                npad = slotw[e]
                groups = _groups(npad)
                if e == 0:
                    # Fine-grained first loads, issued in consumption order and
                    # alternated across the two HWDGE queues (Sync + Scalar) so
                    # the ~600ns per-issue serialization is halved and the first
                    # real matmul can start as soon as possible.
                    g_w0 = wp.tile([128, NH, 128], f16, name="g_w", tag="g_w")
                    u_w0 = wp.tile([128, NH, 128], f16, name="u_w", tag="u_w")
                    xt_tiles = [
                        xp.tile([128, 2, npad], f16, name=f"xt{j}", tag=f"xt{j % 4}")
                        for j in range(8)
                    ]
                    first_loads = [
                        (g_w0[:, 0:4, :], gw[e, 0, :, 0:4, :]),
                        (xt_tiles[0], xTs[e][:, 0:2, :]),
                        (xt_tiles[1], xTs[e][:, 2:4, :]),
                        (g_w0[:, 4:8, :], gw[e, 0, :, 4:8, :]),
                        (xt_tiles[2], xTs[e][:, 4:6, :]),
                        (xt_tiles[3], xTs[e][:, 6:8, :]),
                        (g_w0[:, 8:12, :], gw[e, 0, :, 8:12, :]),
                        (xt_tiles[4], xTs[e][:, 8:10, :]),
                        (xt_tiles[5], xTs[e][:, 10:12, :]),
                        (g_w0[:, 12:16, :], gw[e, 0, :, 12:16, :]),
                        (xt_tiles[6], xTs[e][:, 12:14, :]),
                        (xt_tiles[7], xTs[e][:, 14:16, :]),
                        (u_w0[:, 0:4, :], uw[e, 0, :, 0:4, :]),
                        (u_w0[:, 4:8, :], uw[e, 0, :, 4:8, :]),
                        (u_w0[:, 8:12, :], uw[e, 0, :, 8:12, :]),
                        (u_w0[:, 12:16, :], uw[e, 0, :, 12:16, :]),
                    ]
                    for k, (dst, src) in enumerate(first_loads):
                        eng = nc.sync if k % 2 == 0 else nc.scalar
                        eng.dma_start(out=dst, in_=src)
                    xts = [xt_tiles[h // 2][:, h % 2, :] for h in range(NH)]
                    g0_aps = [g_w0[:, h, :] for h in range(NH)]
                    u0_aps = [u_w0[:, h, :] for h in range(NH)]
                else:
                    g_w0 = wp.tile([128, NH, 128], f16, name="g_w", tag="g_w")
                    u_w0 = wp.tile([128, NH, 128], f16, name="u_w", tag="u_w")
                    xts = []
                    nc.sync.dma_start(out=g_w0, in_=gw[e, 0, :, :, :])
                    for j in range(4):
                        xt_t = xp.tile([128, 4, npad], f16, name=f"xt{j}", tag=f"xt{j}")
                        eng = nc.sync if j % 2 == 0 else nc.scalar
                        eng.dma_start(
                            out=xt_t, in_=xTs[e][:, 4 * j : 4 * (j + 1), :]
                        )
                        xts.extend(xt_t[:, jj, :] for jj in range(4))
                        if j == 0:
                            nc.scalar.dma_start(out=u_w0, in_=uw[e, 0, :, :, :])
                    g0_aps = [g_w0[:, h, :] for h in range(NH)]
                    u0_aps = [u_w0[:, h, :] for h in range(NH)]
                hms = [
                    hp.tile([128, npad], f16, name=f"hm{i}", tag=f"hm{i}")
                    for i in range(NI)
                ]
                for i in range(NI):
                    if i == 0:
                        g_aps, u_aps = g0_aps, u0_aps
                    else:
                        g_w = wp.tile([128, NH, 128], f16, name="g_w", tag="g_w")
                        u_w = wp.tile([128, NH, 128], f16, name="u_w", tag="u_w")
                        nc.sync.dma_start(out=g_w, in_=gw[e, i, :, :, :])
                        nc.scalar.dma_start(out=u_w, in_=uw[e, i, :, :, :])
                        g_aps = [g_w[:, h, :] for h in range(NH)]
                        u_aps = [u_w[:, h, :] for h in range(NH)]
                    if e == 0 and i == 0 and len(groups) == 2:
                        # Interleave the two groups' accumulation sweeps so the
                        # first expert consumes X chunks at half rate, matching
                        # the cold DMA queues' delivery pace.
                        psgs = [
                            pp.tile([128, g1 - g0, ], f32, name="psg", tag="psg", bufs=3)
                            for g0, g1 in groups
                        ]
                        psus = [
                            pp.tile([128, g1 - g0], f32, name="psu", tag="psu", bufs=2)
                            for g0, g1 in groups
                        ]
                        for h in range(NH):
                            for gi, (g0, g1) in enumerate(groups):
                                nc.tensor.matmul(
                                    psgs[gi],
                                    g_aps[h],
                                    xts[h][:, g0:g1],
                                    start=(h == 0),
                                    stop=(h == NH - 1),
                                )
                        for h in range(NH):
                            for gi, (g0, g1) in enumerate(groups):
                                nc.tensor.matmul(
                                    psus[gi],
                                    u_aps[h],
                                    xts[h][:, g0:g1],
                                    start=(h == 0),
                                    stop=(h == NH - 1),
                                )
                        for gi, (g0, g1) in enumerate(groups):
                            sil = tp.tile([128, g1 - g0], f32, name="sil", tag="sil")
                            nc.scalar.activation(sil, psgs[gi], SILU)
                            nc.vector.tensor_mul(hms[i][:, g0:g1], sil, psus[gi])
                        continue_groups = False
                    else:
                        continue_groups = True
                    if continue_groups:
                        for g0, g1 in groups:
                            wdt = g1 - g0
                            psg = pp.tile([128, wdt], f32, name="psg", tag="psg", bufs=3)
                            psu = pp.tile([128, wdt], f32, name="psu", tag="psu", bufs=2)
                            for h in range(NH):
                                nc.tensor.matmul(
                                    psg,
                                    g_aps[h],
                                    xts[h][:, g0:g1],
                                    start=(h == 0),
                                    stop=(h == NH - 1),
                                )
                            for h in range(NH):
                                nc.tensor.matmul(
                                    psu,
                                    u_aps[h],
                                    xts[h][:, g0:g1],
                                    start=(h == 0),
                                    stop=(h == NH - 1),
                                )
                            sil = tp.tile([128, wdt], f32, name="sil", tag="sil")
                            nc.scalar.activation(sil, psg, SILU)
                            nc.vector.tensor_mul(hms[i][:, g0:g1], sil, psu)
                d_w = wp.tile([128, NH, NI, 128], f16, name="d_w", tag="d_w", bufs=2)
                nc.sync.dma_start(out=d_w[:, 0:8, :, :], in_=dw[e, :, 0:8, :, :])
                nc.scalar.dma_start(out=d_w[:, 8:16, :, :], in_=dw[e, :, 8:16, :, :])
                # Output stores: quarters normally; for the last expert use 2-h
                # chunks so the tail after the final matmul is only one small
                # CAST + DMA instead of a 4-h store.
                hchunk = 2 if e == EPC - 1 else 4
                for q in range(NH // hchunk):
                    yt = op.tile(
                        [128, hchunk, npad], f16, name=f"yt{q % 4}", tag=f"yt{q % 4}"
                    )
                    for hh in range(hchunk):
                        h = hchunk * q + hh
                        for gi, (g0, g1) in enumerate(groups):
                            wdt = g1 - g0
                            pso = pp.tile([128, wdt], f32, name="pso", tag="pso")
                            for i in range(NI):
                                nc.tensor.matmul(
                                    pso,
                                    d_w[:, h, i, :],
                                    hms[i][:, g0:g1],
                                    start=(i == 0),
                                    stop=(i == NI - 1),
                                )
                            nc.vector.tensor_copy(yt[:, hh, g0:g1], pso)
                    eng = nc.sync if q % 2 == 0 else nc.scalar
                    eng.dma_start(
                        out=yTs[e][:, hchunk * q : hchunk * (q + 1), :], in_=yt
                    )
    nc.compile()
    return nc


def _install_neff_cache():
    """Cache walrus NEFF compiles on disk keyed by BIR hash (compile of the
    ~11k-instruction program takes minutes; the BIR is deterministic)."""
    import hashlib
    import shutil

    import concourse.bass2jax as bass2jax
    from concourse.bass_utils import compile_bir_kernel as _orig

    if getattr(bass2jax.compile_bir_kernel, "_moe_cached", False):
        return
    cache_dir = os.environ.get("BASS_NEFF_CACHE", "/tmp/bass_neff_cache")
    try:
        os.makedirs(cache_dir, exist_ok=True)
    except OSError:
        return

    def cached(bir_json, tmpdir, neff_name="file.neff"):
        key = hashlib.sha256(bir_json).hexdigest()[:24]
        cpath = os.path.join(cache_dir, key + ".neff")
        dst = os.path.join(tmpdir, neff_name)
        if os.path.exists(cpath):
            shutil.copy(cpath, dst)
            return dst
        out = _orig(bir_json, tmpdir, neff_name)
        try:
            shutil.copy(out, cpath)
        except OSError:
            pass
        return out

    cached._moe_cached = True
    bass2jax.compile_bir_kernel = cached


def _install_ntff_hook_shim():
    """Provide antenv.axon_hooks (absent in this container) so that
    run_bass_kernel_spmd(trace=True) can capture NTFF profiles via the
    axon .so — mirrors trn_agent_boot.trn_boot's ctypes hook."""
    import types
    import ctypes
    import contextlib

    if "antenv.axon_hooks" in sys.modules:
        return
    so_path = "/opt/axon/libaxon_pjrt.so"
    lib = ctypes.CDLL(so_path)
    if not hasattr(lib, "axon_start_nrt_profile"):
        return
    lib.axon_start_nrt_profile.argtypes = [
        ctypes.POINTER(ctypes.c_int64),
        ctypes.c_size_t,
    ]
    lib.axon_start_nrt_profile.restype = ctypes.c_int64
    lib.axon_stop_nrt_profile.argtypes = [ctypes.c_char_p]
    lib.axon_stop_nrt_profile.restype = ctypes.c_int64

    @contextlib.contextmanager
    def _hook(output_dir, device_ids):
        import jax

        jax.devices()
        if device_ids:
            ids = (ctypes.c_int64 * len(device_ids))(*device_ids)
            rc = lib.axon_start_nrt_profile(ids, len(device_ids))
        else:
            rc = lib.axon_start_nrt_profile(None, 0)
        if rc != 0:
            raise RuntimeError(f"axon_start_nrt_profile rc={rc}")
        try:
            yield
        finally:
            n = lib.axon_stop_nrt_profile(str(output_dir).encode())
            print(f"profile: {n} file(s) written to {output_dir}", file=sys.stderr)

    mod = types.ModuleType("antenv.axon_hooks")
    mod.get_axon_ntff_profile_hook = lambda: _hook
    mod.set_axon_ntff_profile_hook = lambda h: None
    sys.modules["antenv.axon_hooks"] = mod


def kernel(
    hidden_states,
    routing_weights,
    selected_experts,
    gate_proj,
    up_proj,
    down_proj,
):
    global LAST_EXEC_NS, LAST_RESULTS
    from concourse.bass_utils import run_bass_kernel_spmd

    _install_neff_cache()

    x = np.ascontiguousarray(np.asarray(hidden_states, dtype=np.float32))
    rw = np.asarray(routing_weights, dtype=np.float32)
    sel = np.asarray(selected_experts).astype(np.int64)
    gate = np.asarray(gate_proj, dtype=np.float32)
    up = np.asarray(up_proj, dtype=np.float32)
    down = np.asarray(down_proj, dtype=np.float32)

    # ---- host dispatch (mirrors reference's stable sort-by-expert) ----
    flat_e = sel.reshape(-1)
    order = np.argsort(flat_e, kind="stable")
    sorted_e = flat_e[order]
    counts = np.bincount(flat_e, minlength=E)
    offsets = np.concatenate([[0], np.cumsum(counts)[:-1]])
    pos = np.arange(flat_e.shape[0], dtype=np.int64) - offsets[sorted_e]

    # ---- slot assignment: experts ranked by load desc; rank r -> core r%8,
    # slot r//8; slot width = rounded max count in its rank group ----
    rank = np.argsort(-counts, kind="stable")
    perm = rank.reshape(EPC, NCORES).T  # perm[c, j] = expert id
    slotw = tuple(
        int(min(CAP, max(16, -(-int(counts[perm[:, j]].max()) // 2) * 2)))
        for j in range(EPC)
    )
    w_of_expert = np.zeros(E, dtype=np.int64)
    for c in range(NCORES):
        for j in range(EPC):
            w_of_expert[perm[c, j]] = slotw[j]

    keep = pos < w_of_expert[sorted_e]  # width >= min(count, CAP); drops only > CAP

    tok = order // K
    ke = sorted_e[keep]
    kp = pos[keep]

    # Dense per-expert buffers, transposed: xbuf[e] = X_e  [w_e, H]
    maxw = max(slotw)
    xbuf = np.zeros((E, maxw, H), dtype=np.float32)
    xbuf[ke, kp] = x[tok[keep]]

    # ---- weight layouts (contiguous per-DMA blocks) ----
    # gate/up slice for (e, i): [128p, NH, 128c] where [p, h, c] = W[h*128+p, i*128+c]
    gate_r = gate.reshape(E, NH, 128, NI, 128).transpose(0, 3, 2, 1, 4)
    up_r = up.reshape(E, NH, 128, NI, 128).transpose(0, 3, 2, 1, 4)
    # down block for e: [128p, NH, NI, 128m] where [p, h, i, m] = W[i*128+p, h*128+m]
    down_r = down.reshape(E, NI, 128, NH, 128).transpose(0, 2, 3, 1, 4)

    nc = _prog_cache.get(slotw)
    if nc is None:
        nc = _build_program(slotw)
        _prog_cache[slotw] = nc

    in_maps = []
    for c in range(NCORES):
        m = {
            "wz": np.zeros((128, 64), dtype=np.float16),
            "gw": np.ascontiguousarray(gate_r[perm[c]], dtype=np.float16),
            "uw": np.ascontiguousarray(up_r[perm[c]], dtype=np.float16),
            "dw": np.ascontiguousarray(down_r[perm[c]], dtype=np.float16),
        }
        for j in range(EPC):
            e = perm[c, j]
            w = slotw[j]
            # [w, H] -> [128p, NH, w]
            m[f"xT{j}"] = np.ascontiguousarray(
                xbuf[e, :w].T.reshape(NH, 128, w).transpose(1, 0, 2),
                dtype=np.float16,
            )
        in_maps.append(m)

    trace = bool(os.environ.get("BASS_MOE_TRACE"))
    kwargs = {}
    if trace:
        _install_ntff_hook_shim()
        tcores = os.environ.get("BASS_MOE_TRACE_CORES", "0")
        kwargs = dict(trace=True, trace_cores=[int(c) for c in tcores.split(",")])
    res = run_bass_kernel_spmd(nc, in_maps, core_ids=list(range(NCORES)), **kwargs)
    LAST_EXEC_NS = res.exec_time_ns
    LAST_RESULTS = res

    # ---- host combine ----
    # per expert e at (core c, slot j): yT{j} is [128p, NH, w] = O_e^T blocks
    o_all = np.zeros((E, maxw, H), dtype=np.float32)
    for c in range(NCORES):
        for j in range(EPC):
            e = perm[c, j]
            w = slotw[j]
            o_all[e, :w] = (
                res.results[c][f"yT{j}"]
                .astype(np.float32)
                .transpose(2, 1, 0)
                .reshape(w, H)
            )

    gathered = np.zeros((flat_e.shape[0], H), dtype=np.float32)
    gathered[order[keep]] = o_all[ke, kp]
    y = (gathered.reshape(T, K, H) * rw[:, :, None]).sum(axis=1, dtype=np.float32)
    return y.astype(np.float32)

